# revision 64
# baseline (speedup 1.0000x reference)
"""CaptioningRNN forward loss on 8 Trainium2 NeuronCores.

Data-parallel over N: each core runs 16 of the 128 sequences end-to-end;
the scalar loss partials are summed on the host.

v5 design (Tile-scheduler-aware; from v2-v4 NTFF traces):
- The Tile scheduler reorders instructions by readiness, so ordering is
  controlled only through real dependencies and buffer rings:
  - wv (W_vocab fp8) DMA packs are dependency-gated on early scan steps
    via 1-element dummy copies, so the 17MB stream cannot starve the
    critical Wh/Wx/W_proj DMAs the head needs.
  - vocab-group pacing matches wv arrival: 1 group/step before b=16,
    2/step after, remainder as a dense tail.
- featT, the 128x128 identity, and the vocab iota are host-prepared DMA
  consts: no featT PE transposes, no GpSimd iota (the gathers own the
  GpSimd queue), 7 big DMA starts instead of 22.
- p3 PSUM ring has 3 bufs (xW shares its tag) so DR groups don't stall
  on exp reading PSUM; exp_s/stt_s scratch have separate rings.
- Per-step tanh is 2 halves; the merged FD=128 identity matmul seeds the
  step's PSUM with xW.
- Target extraction reads the fp16 exp copy and accumulates exp(s_y);
  loss = sum mask*(ln(s_red) - ln(t_red)), so Ln is first needed after
  the last tanh and the ln+exp act table loads once, off the critical
  tail.

Shapes (hardcoded): N=128, T=33 (32 steps), Dfeat=512, W=512, H=1024,
V=16384.
"""
import numpy as np
import concourse.bass as bass
import concourse.tile as tile
from concourse import bacc, mybir
from concourse.bass_utils import run_bass_kernel_spmd
from contextlib import ExitStack

dt = mybir.dt
AF = mybir.ActivationFunctionType
OP = mybir.AluOpType
PM = mybir.MatmulPerfMode

N_CORES = 8
NL = 16          # sequences per core
T_STEPS = 32     # scan steps (T-1)
DF = 512         # feature dim
WD = 512         # word vec dim
H = 1024         # hidden dim
V = 16384        # vocab
NTOK = NL * T_STEPS          # 512 tokens per core (t-major: tok = t*16 + n)
NG = NTOK // 128             # 4 groups of 128 tokens
NJ2 = V // 1024              # 16 vocab column tiles (1024 wide, 2 PSUM banks)
OC = H // 128                # 8 hidden-chunk tiles
KC8 = H // 256               # 4 fp8 DoubleRow contraction chunks
KW = WD // 128               # 4 contraction chunks over W
HCOLS = (T_STEPS + 1) * NL   # 528 hT columns (h0 + 32 steps)
NPACK = 4                    # wv DMA packs (4 jj tiles each)
WVGATE = [0, 1, 3, 5]        # scan step whose tanh gates wv pack p's DMA
MW = NJ2 + 2                 # meta width: NJ2 yrel cols + tok + maskn

_nc_cache = {}


def build_program(nobias=False):
    import os
    kloop = int(os.environ.get("KLOOP", "1"))  # HW-loop reps for timing
    key = (kloop, nobias)
    if key in _nc_cache:
        return _nc_cache[key]
    nc = bacc.Bacc("TRN2", target_bir_lowering=False, debug=False,
                   num_devices=N_CORES, num_swdge_queues=4)

    # ---- DRAM parameters (per-core shards / replicated weights) ----
    meta_d = nc.dram_tensor("meta", [128, NG, MW], dt.float32,
                            kind="ExternalInput")
    id128_d = nc.dram_tensor("id128b", [128, 128], dt.bfloat16,
                             kind="ExternalInput")
    featT_d = nc.dram_tensor("featT", [128, KW, NL], dt.bfloat16,
                             kind="ExternalInput")
    xT_d = nc.dram_tensor("xT", [128, KW, NTOK], dt.bfloat16,
                          kind="ExternalInput")
    wproj_d = nc.dram_tensor("W_proj", [128, KW, H], dt.bfloat16,
                             kind="ExternalInput")
    wx_d = nc.dram_tensor("Wx", [128, KW, H], dt.bfloat16, kind="ExternalInput")
    wh_d = nc.dram_tensor("Wh", [128, OC, H], dt.bfloat16, kind="ExternalInput")
    iota_d = nc.dram_tensor("iota16", [128, 2, 512], dt.float16,
                            kind="ExternalInput")
    if not nobias:
        bprojT_d = nc.dram_tensor("bprojT", [128, OC], dt.float32,
                                  kind="ExternalInput")
        brnn_d = nc.dram_tensor("b_rnn", [1, H], dt.bfloat16,
                                kind="ExternalInput")
        bvoc_d = nc.dram_tensor("b_vocab", [1, NJ2, 2, 1024], dt.float8e4,
                                kind="ExternalInput")
    wv_d = nc.dram_tensor("WV8", [NPACK, 128, NJ2 // NPACK, 8192],
                          dt.float8e4, kind="ExternalInput")
    loss_d = nc.dram_tensor("loss", [1, 4], dt.float32, kind="ExternalOutput")

    with tile.TileContext(nc) as tc, ExitStack() as ctx:
        if kloop > 1:
            ctx.enter_context(tc.For_i(0, kloop, 1))
        const = ctx.enter_context(tc.tile_pool(name="const", bufs=1))
        acts = ctx.enter_context(tc.tile_pool(name="acts", bufs=1))
        wts = ctx.enter_context(tc.tile_pool(name="wts", bufs=1))
        scr = ctx.enter_context(tc.tile_pool(name="scr", bufs=2))

        # ---- DMAs: critical path spread over the two HWDGE queues (Sync,
        # Scalar), ordered by first use ----
        meta_t = const.tile([128, NG, MW], dt.float32)
        nc.sync.dma_start(meta_t[:], meta_d.ap())
        ident128b = const.tile([128, 128], dt.bfloat16)
        nc.sync.dma_start(ident128b[:], id128_d.ap())
        featT = const.tile([128, KW, NL], dt.bfloat16)
        nc.sync.dma_start(featT[:], featT_d.ap())
        xT_t = acts.tile([128, KW, NTOK], dt.bfloat16)
        nc.sync.dma_start(xT_t[:], xT_d.ap())
        wx_t = wts.tile([128, KW, H], dt.bfloat16)
        nc.sync.dma_start(wx_t[:], wx_d.ap())
        wh_t = wts.tile([128, OC, H], dt.bfloat16)
        nc.sync.dma_start(wh_t[:], wh_d.ap())
        wp_t = wts.tile([128, KW, H], dt.bfloat16)
        nc.scalar.dma_start(wp_t[:], wproj_d.ap())
        iota16 = const.tile([128, 2, 512], dt.float16)
        nc.scalar.dma_start(iota16[:], iota_d.ap())

        def wh_chunk(kc, oc):
            return wh_t[:, kc, oc * 128:(oc + 1) * 128]

        def xT_chunk(g, kc):
            return xT_t[:, kc, g * 128:(g + 1) * 128]
        if not nobias:
            bprojT_t = const.tile([128, OC], dt.float32)
            nc.sync.dma_start(bprojT_t[:], bprojT_d.ap())
            brnn_t = const.tile([1, H], dt.bfloat16)
            nc.sync.dma_start(brnn_t[:], brnn_d.ap())
            bvoc_t = acts.tile([1, NJ2, 2, 1024], dt.float8e4)
            nc.sync.dma_start(bvoc_t[:], bvoc_d.ap())

        yrel_t = meta_t[:, :, 0:NJ2]                          # [128, NG, NJ2]

        onescol = const.tile([128, 1], dt.float32)
        nc.vector.memset(onescol[:], 1.0)
        if not nobias:
            ones_f = const.tile([1, 512], dt.float32)
            nc.vector.memset(ones_f[:], 1.0)
            ones_row = const.tile([1, 512], dt.bfloat16)
            nc.vector.tensor_copy(ones_row[:], ones_f[:])
            ones8 = const.tile([1, 2, 128], dt.float8e4)
            nc.vector.tensor_copy(ones8[:, 0, :], ones_f[:, :128])
            nc.vector.tensor_copy(ones8[:, 1, :], ones_f[:, :128])

        # ---- persistent activations ----
        hT_all = acts.tile([128, OC, HCOLS], dt.bfloat16)   # h transposed, bf16
        hT8_all = acts.tile([128, OC, HCOLS], dt.float8e4)  # fp8 copy for p3
        xw_sb = acts.tile([128, OC, NTOK], dt.bfloat16)     # xW^T (+ b_rnn)
        s_cols = acts.tile([128, NG, NJ2], dt.float32)      # exp-sum partials
        t_cols = acts.tile([128, NG, NJ2], dt.float32)      # exp(s_y) partials
        masked = acts.tile([128, NG], dt.float32)           # per-m masked loss

        # wv pack tiles; their dma_starts are dependency-gated below so the
        # scheduler cannot float the 17MB ahead of the critical weights
        # wv packs: dma_starts are issued inside the scan loop, gated on
        # an early scan step's tanh via a dummy 1-elem copy (WAW with the
        # pack DMA), so the 17MB cannot starve the critical weights
        wvp = ctx.enter_context(tc.tile_pool(name="wvp", bufs=1))
        wv_packs = [wvp.tile([128, NJ2 // NPACK, 8192], dt.float8e4,
                             tag=f"wv{p}", name=f"wv{p}")
                    for p in range(NPACK)]

        def start_wv(p):
            gate_col = (WVGATE[p] + 1) * NL
            nc.vector.tensor_copy(wv_packs[p][:1, 0, 0:1],
                                  hT_all[:1, 0, gate_col:gate_col + 1])
            eng = nc.scalar if p % 2 == 0 else nc.sync
            eng.dma_start(wv_packs[p][:], wv_d.ap()[p])

        # ---- PE phase 1: h0 ----
        with tc.tile_pool(name="psE", bufs=2, space="PSUM") as psE:
            for oc in range(OC):
                ps_h = psE.tile([128, 16], dt.float32, space="PSUM", tag="h0")
                for kc in range(KW):
                    nc.tensor.matmul(
                        out=ps_h[:],
                        lhsT=wp_t[:, kc, oc * 128:(oc + 1) * 128],
                        rhs=featT[:, kc, :],
                        start=(kc == 0), stop=(kc == KW - 1))
                if nobias:
                    nc.scalar.copy(hT_all[:, oc, 0:NL], ps_h[:])
                else:
                    nc.scalar.add(hT_all[:, oc, 0:NL], ps_h[:],
                                  bprojT_t[:, oc:oc + 1])
            nc.vector.tensor_copy(hT8_all[:, :, 0:NL], hT_all[:, :, 0:NL])

        # ---- main loop ----
        with tc.tile_pool(name="psS", bufs=2, space="PSUM") as psS, \
             tc.tile_pool(name="psB", bufs=3, space="PSUM") as psB:

            def emit_xw(g, oc):
                """xW^T chunk for token group g, output rows oc."""
                ps = psB.tile([128, 2, 512], dt.float32, space="PSUM",
                              tag="big")
                for kc in range(KW):
                    nc.tensor.matmul(
                        out=ps[:, 0, 0:128],
                        lhsT=wx_t[:, kc, oc * 128:(oc + 1) * 128],
                        rhs=xT_chunk(g, kc),
                        start=(kc == 0), stop=(nobias and kc == KW - 1))
                if not nobias:
                    nc.tensor.matmul(
                        out=ps[:, 0, 0:128],
                        lhsT=brnn_t[:, oc * 128:(oc + 1) * 128],
                        rhs=ones_row[:, :128],
                        start=False, stop=True)
                nc.vector.tensor_copy(
                    xw_sb[:, oc, g * 128:(g + 1) * 128], ps[:, 0, 0:128])

            def emit_step(b):
                lo, hi = b * NL, (b + 1) * NL
                ps_step = psS.tile([128, OC, NL], dt.float32, space="PSUM",
                                   tag="scan")
                nc.tensor.matmul(out=ps_step[:], lhsT=ident128b[:],
                                 rhs=xw_sb[:, :, lo:hi],
                                 start=True, stop=False)
                for oc in range(OC):
                    for kc in range(OC):
                        nc.tensor.matmul(
                            out=ps_step[:, oc, :],
                            lhsT=wh_chunk(kc, oc),
                            rhs=hT_all[:, kc, lo:hi],
                            start=False, stop=(kc == OC - 1))
                for half in range(2):
                    o0, o1 = half * 4, (half + 1) * 4
                    nc.scalar.activation(hT_all[:, o0:o1, hi:hi + NL],
                                         ps_step[:, o0:o1, :], AF.Tanh)
                nc.vector.tensor_copy(hT8_all[:, :, hi:hi + NL],
                                      hT_all[:, :, hi:hi + NL])

            def emit_group(jj, m):
                wv_t = wv_packs[jj // (NJ2 // NPACK)]
                t0 = NL + m * 128
                ps = psB.tile([128, 2, 512], dt.float32, space="PSUM",
                              tag="big")
                for half in range(2):
                    for kc in range(KC8):
                        base = (half * KC8 + kc) * 1024
                        nc.tensor.matmul(
                            out=ps[:, half, :],
                            lhsT=hT8_all[:, 2 * kc:2 * kc + 2, t0:t0 + 128],
                            rhs=wv_t[:, jj % (NJ2 // NPACK), base:base + 1024]
                            .rearrange("p (k d) -> p k d", k=2),
                            start=(kc == 0),
                            stop=(nobias and kc == KC8 - 1),
                            perf_mode=PM.DoubleRow)
                    if not nobias:
                        nc.tensor.matmul(
                            out=ps[:, half, :], lhsT=ones8[:],
                            rhs=bvoc_t[:, jj, :, half * 512:(half + 1) * 512],
                            start=False, stop=True,
                            perf_mode=PM.DoubleRow)
                exp_s = scr.tile([128, 2, 512], dt.float16, tag="se")
                nc.scalar.activation(exp_s[:], ps[:], AF.Exp,
                                     accum_out=s_cols[:, m, jj:jj + 1])
                stt_s = scr.tile([128, 2, 512], dt.float16, tag="st")
                nc.vector.scalar_tensor_tensor(
                    out=stt_s[:], in0=iota16[:], scalar=yrel_t[:, m, jj:jj + 1],
                    in1=exp_s[:], op0=OP.is_equal, op1=OP.mult,
                    accum_out=t_cols[:, m, jj:jj + 1])

            def emit_loss_m(m):
                """masked[:, m] = maskn * (ln(s_red) - ln(t_red))."""
                s_red = scr.tile([128, 1], dt.float32, tag="sr")
                nc.vector.tensor_reduce(out=s_red[:], in_=s_cols[:, m, :],
                                        axis=mybir.AxisListType.X, op=OP.add)
                t_red = scr.tile([128, 1], dt.float32, tag="tr")
                nc.vector.tensor_reduce(out=t_red[:], in_=t_cols[:, m, :],
                                        axis=mybir.AxisListType.X, op=OP.add)
                ln_s = scr.tile([128, 1], dt.float32, tag="ls")
                nc.scalar.activation(ln_s[:], s_red[:], AF.Ln)
                ln_t = scr.tile([128, 1], dt.float32, tag="lt")
                nc.scalar.activation(ln_t[:], t_red[:], AF.Ln)
                diff = scr.tile([128, 1], dt.float32, tag="df")
                nc.vector.tensor_tensor(out=diff[:], in0=ln_s[:],
                                        in1=ln_t[:], op=OP.subtract)
                nc.vector.tensor_tensor(out=masked[:, m:m + 1], in0=diff[:],
                                        in1=meta_t[:, m, NJ2 + 1:NJ2 + 2],
                                        op=OP.mult)

            for g in range(NG):
                for oc in range(OC):
                    emit_xw(g, oc)
            todo = [(jj, m) for m in range(NG) for jj in range(NJ2)]

            for b in range(T_STEPS):
                if b >= 16 and todo and (todo[0][1] + 1) * 8 <= b:
                    emit_group(*todo.pop(0))
                emit_step(b)
                if b in WVGATE:
                    start_wv(WVGATE.index(b))
                if todo and (todo[0][1] + 1) * 8 <= b + 1:
                    emit_group(*todo.pop(0))

            # loss partials for fully-accumulated m's (the ln+exp act table
            # loads here, after the last tanh, off the critical tail)
            pending_m = {m for _, m in todo}
            for m in range(NG):
                if m not in pending_m:
                    emit_loss_m(m)
            # leftover vocab groups, dense DR stream
            for jj, m in todo:
                emit_group(jj, m)
            for m in sorted(pending_m):
                emit_loss_m(m)

            # ---- final: sum masked over partitions and columns ----
            ps_l = psB.tile([128, 2, 512], dt.float32, space="PSUM", tag="big")
            nc.tensor.matmul(out=ps_l[:1, 0, :NG], lhsT=onescol[:],
                             rhs=masked[:], start=True, stop=True)
            lsb = acts.tile([1, 4], dt.float32)
            nc.vector.tensor_copy(lsb[:], ps_l[:1, 0, :NG])
            lfin = acts.tile([1, 4], dt.float32)
            nc.vector.memset(lfin[:], 0.0)
            nc.vector.tensor_reduce(out=lfin[:, :1], in_=lsb[:],
                                    axis=mybir.AxisListType.X, op=OP.add)
            nc.sync.dma_start(loss_d.ap(), lfin[:])

    nc.compile()
    _nc_cache[key] = nc
    return nc


def make_in_maps(features, captions, W_proj, b_proj, W_embed, Wx, Wh, b,
                 W_vocab, b_vocab, nobias=False):
    bf16 = dt.np(dt.bfloat16)
    f8 = dt.np(dt.float8e4)
    features = np.asarray(features, dtype=np.float32)
    cap = np.asarray(captions).astype(np.int64)
    # Wv [H, V] -> [NPACK, 128, NJ2/NPACK, 8192]: DR pair (p, r) <-> H row
    # (2kc+r)*128+p; vocab tiled as 16 x (2 halves x 512)
    wv8 = (np.asarray(W_vocab, dtype=np.float32)
           .reshape(KC8, 2, 128, NJ2, 2, 512).transpose(3, 2, 4, 0, 1, 5)
           .reshape(NPACK, NJ2 // NPACK, 128, 8192)
           .transpose(0, 2, 1, 3).astype(f8))
    wembed_f = np.asarray(W_embed, dtype=np.float32).astype(bf16).astype(np.float32)
    shared = {
        "id128b": np.eye(128, dtype=np.float32).astype(bf16),
        "iota16": np.ascontiguousarray(
            np.broadcast_to(
                np.arange(1024, dtype=np.float16).reshape(1, 2, 512),
                (128, 2, 512))),
        "W_proj": np.ascontiguousarray(
            np.asarray(W_proj, np.float32).reshape(KW, 128, H).transpose(1, 0, 2)
        ).astype(bf16),
        "Wx": np.ascontiguousarray(
            np.asarray(Wx, np.float32).reshape(KW, 128, H).transpose(1, 0, 2)
        ).astype(bf16),
        "Wh": np.ascontiguousarray(
            np.asarray(Wh, np.float32).reshape(OC, 128, H).transpose(1, 0, 2)
        ).astype(bf16),
        "WV8": np.ascontiguousarray(wv8),
    }
    if not nobias:
        bv8 = np.zeros((1, NJ2, 2, 1024), dtype=np.float32)
        bv8[0, :, 0, :] = np.asarray(b_vocab, dtype=np.float32).reshape(NJ2, 1024)
        shared["b_vocab"] = bv8.astype(f8)
        shared["bprojT"] = np.ascontiguousarray(
            np.asarray(b_proj, dtype=np.float32).reshape(OC, 128).T)
        shared["b_rnn"] = np.asarray(b, dtype=np.float32).reshape(1, H).astype(bf16)
    in_maps = []
    for c in range(N_CORES):
        capc = cap[c * NL:(c + 1) * NL]              # (16, 33)
        tok_tm = capc[:, :T_STEPS].T.reshape(NTOK)   # token ids, t-major
        y_tm = capc[:, 1:].T.reshape(NTOK)           # targets, t-major
        tok_pg = tok_tm.reshape(NG, 128).T.astype(np.int32)         # (128, NG)
        y_pg = y_tm.reshape(NG, 128).T                               # (128, NG)
        yrel = (y_pg[:, :, None].astype(np.float32)
                - (np.arange(NJ2, dtype=np.float32) * 1024)[None, None, :])
        maskn = (y_pg != 0).astype(np.float32) / 128.0
        meta = np.zeros((128, NG, MW), dtype=np.float32)
        meta[:, :, 0:NJ2] = yrel
        meta[:, :, NJ2 + 1] = maskn
        # host-side embedding gather, already transposed: xT[p, kc, t*16+n]
        # = W_embed[cap[n, t], kc*128 + p] (in bf16 precision)
        x = wembed_f[capc[:, :T_STEPS]]                      # (16, 32, 512)
        xT = np.ascontiguousarray(
            x.reshape(NL, T_STEPS, KW, 128).transpose(3, 2, 1, 0)
            .reshape(128, KW, NTOK)).astype(bf16)
        featT = np.ascontiguousarray(
            features[c * NL:(c + 1) * NL].T.reshape(KW, 128, NL)
            .transpose(1, 0, 2)).astype(bf16)
        in_maps.append({
            "featT": featT,
            "meta": meta,
            "xT": xT,
            **shared,
        })
    return in_maps


def prepare(inputs):
    nobias = (not np.any(np.asarray(inputs["b_vocab"]))
              and not np.any(np.asarray(inputs["b"]))
              and not np.any(np.asarray(inputs["b_proj"])))
    nc = build_program(nobias=nobias)
    in_maps = make_in_maps(**inputs, nobias=nobias)
    return nc, in_maps


def kernel(**inputs) -> np.ndarray:
    nc, in_maps = prepare(inputs)
    res = run_bass_kernel_spmd(nc, in_maps, list(range(N_CORES)))
    return np.float32(sum(res.results[c]["loss"][0, 0] for c in range(N_CORES)))
